# revision 18
# baseline (speedup 1.0000x reference)
"""Trainium2 Bass kernel for a GQA attention block (B=2, L=2048, D=2048,
16 q-heads / 8 kv-heads, head_dim=128), sharded over 8 NeuronCores.

Sharding: core c -> batch b = c // 4, head-group g = c % 4 (4 q-heads and
their 2 kv-heads).  Each core computes its heads' attention plus the partial
output projection; the host sums the 4 partials per batch.

Self-contained: only needs numpy / ml_dtypes / concourse (on PYTHONPATH in
this container).
"""

import math

import numpy as np
import ml_dtypes

import concourse.bass as bass
import concourse.bass2jax as bass2jax
import concourse.mybir as mybir
import concourse.tile as tile
from concourse.bass_utils import run_bass_kernel_spmd
from concourse.vector_clock import ScopedClock, VectorClock


def _legalize_bir_waits(bir_bytes):
    """This walrus build supports only ONE sync-wait slot per instruction.
    Hoist extra waits onto NoOp instructions inserted just before the
    offender (same engine, so the engine stream still blocks in order)."""
    import orjson

    d = orjson.loads(bir_bytes)
    n_split = 0
    for f in d["functions"]:
        for bb in f["blocks"]:
            out = []
            for inst in bb["instructions"]:
                si = inst.get("sync_info")
                waits = (si or {}).get("on_wait") or []
                if len(waits) > 1:
                    for j, w in enumerate(waits[:-1]):
                        n_split += 1
                        out.append({
                            "engine": inst["engine"], "ins": [], "outs": [],
                            "name": f"{inst['name']}__w{j}",
                            "opcode": "NoOp",
                            "sync_info": {"on_wait": [w], "on_update": []},
                        })
                    si["on_wait"] = [waits[-1]]
                out.append(inst)
            bb["instructions"] = out
    return orjson.dumps(d)


_orig_compile_bir_kernel = bass2jax.compile_bir_kernel


def _patched_compile_bir_kernel(ant_bir_str, *args, **kwargs):
    return _orig_compile_bir_kernel(_legalize_bir_waits(ant_bir_str), *args, **kwargs)


bass2jax.compile_bir_kernel = _patched_compile_bir_kernel

BF16 = mybir.dt.bfloat16
F32 = mybir.dt.float32

# Full-problem constants
B, L, D = 2, 2048, 2048
N_HEADS, N_KV, H = 16, 8, 128
EPS = 1e-6
ROPE_THETA = 1e6
N_CORES = 8
QH_PER_CORE = N_HEADS // (N_CORES // B)   # 4
KV_PER_CORE = N_KV // (N_CORES // B)      # 2
SCALE = H ** -0.5


class PatchedTileContext(tile.TileContext):
    """This walrus build only supports one sync-wait slot on a CTRL (Drain)
    instruction; split the tail-drain waits across one drain per processor."""

    def _drain_and_barrier(self, tick_clock, wait_clock):
        gc = tick_clock.global_clock
        n = len(gc)
        for p in range(n):
            t = gc[p]
            if t > 0:
                vc = VectorClock([t if i == p else 0 for i in range(n)])
                d = self.nc.sync.drain()
                wait_clock.add_sem_waits(d.ins, ScopedClock({None: vc}))
                si = d.ins.sync_info
                nw = len(si.on_wait) if si is not None else 0
                assert nw <= 1, f"proc {p} produced {nw} waits"
        self.nc.all_engine_barrier()
        assert self.sems is not None
        popped = self.nc._tile_sem_poison_stack.pop()
        assert popped is self._sem_poison
        self.nc.clear_and_free_semaphores(list(self.sems.allocated().values()))
        self.nc.all_engine_barrier()


def build_core_kernel(L_=L, D_=D, nq=QH_PER_CORE, nkv=KV_PER_CORE, causal=True):
    """One core's program.  Inputs (DRAM):
      xT     [D, L]  bf16   x[b].T
      wqkv   [D, nq*H + 2*nkv*H] bf16  ([wq heads | wk heads | wv heads])
      wo     [nq*H, D] bf16
      ropeq  [4, L, H//2] f32  (A,B,C,D tables: cos/sin with q_norm_w folded)
      ropek  [4, L, H//2] f32
      maskT  [L, L] bf16 (only if causal=False; 0/1 multiplicative, [s, l])
    Output:
      out [L, D] f32 — partial sum over this core's heads.
    """
    HH = H // 2
    n_lb = L_ // 128          # L blocks of 128
    n_dc = D_ // 128          # D contraction chunks
    n_lqb = L_ // 512         # q blocks of 512
    QCOLS = nq * H
    KCOLS = nkv * H
    KV_COLS = 2 * nkv * H
    W_COLS = QCOLS + KV_COLS
    assert W_COLS % 512 == 0
    n_wslab = W_COLS // 512   # 512-wide slabs of the qkv projection

    nc = bass.Bass()
    xT_d = nc.dram_tensor("xT", [D_, L_], BF16, kind="ExternalInput")
    wqkv_d = nc.dram_tensor("wqkv", [D_, W_COLS], BF16, kind="ExternalInput")
    wo_d = nc.dram_tensor("wo", [QCOLS, D_], BF16, kind="ExternalInput")
    ropeq_d = nc.dram_tensor("ropeq", [4, L_, HH], F32, kind="ExternalInput")
    ropek_d = nc.dram_tensor("ropek", [4, L_, HH], F32, kind="ExternalInput")
    if not causal:
        maskT_d = nc.dram_tensor("maskT", [L_, L_], BF16, kind="ExternalInput")
    out_d = nc.dram_tensor("out", [L_, D_], F32, kind="ExternalOutput")

    with PatchedTileContext(nc) as tc:
        with (
            tc.tile_pool(name="res", bufs=1) as res,
            tc.tile_pool(name="ropetab", bufs=3) as ropetab,
            tc.tile_pool(name="work", bufs=3) as work,
            tc.tile_pool(name="stats", bufs=6) as stats,
            tc.tile_pool(name="expp", bufs=4) as expp,
            tc.tile_pool(name="outp", bufs=3) as outp,
            tc.tile_pool(name="psum", bufs=1, space="PSUM") as psum,
            tc.tile_pool(name="maskp", bufs=2) as maskp,
            tc.tile_pool(name="dramp", bufs=3, space="DRAM") as dramp,
        ):
            # ---- resident loads ----
            xT_sb = res.tile([128, n_dc, L_], BF16, tag="xT")
            xr = xT_d.rearrange("(dc p) l -> p dc l", p=128)
            step = max(1, n_dc // 4)
            for i in range(0, n_dc, step):
                j = min(n_dc, i + step)
                nc.sync.dma_start(out=xT_sb[:, i:j, :], in_=xr[:, i:j, :])
            wqkv_sb = res.tile([128, n_dc, W_COLS], BF16, tag="wqkv")
            nc.sync.dma_start(
                out=wqkv_sb, in_=wqkv_d.rearrange("(dc p) c -> p dc c", p=128)
            )
            wo_sb = res.tile([128, nq, D_], BF16, tag="wo")
            nc.sync.dma_start(
                out=wo_sb, in_=wo_d.rearrange("(hh p) d -> p hh d", p=128)
            )
            ones_sb = res.tile([128, 1], BF16, tag="ones")
            nc.vector.memset(ones_sb, 1.0)
            eps_sb = res.tile([128, 1], F32, tag="eps")
            nc.vector.memset(eps_sb, EPS)

            v_sb = res.tile([128, n_lb, KCOLS], BF16, tag="v")
            qT_sb = res.tile([128, nq, L_], BF16, tag="qT")
            kT_sb = res.tile([128, nkv, L_], BF16, tag="kT")
            qkvT_sb = res.tile([128, nq, L_], BF16, tag="qkvT")

            # ---- phase B: qkv projection + rmsnorm + rope + transposes ----
            def norm_rope_head(pq, col0, rtab, dest_T, dcol, lb):
                """RMS-norm + rope one head living at pq[:, col0:col0+H];
                write bf16 transpose into dest_T[:, dcol, lb*128:...]."""
                src = pq[:, col0:col0 + H]
                sq = work.tile([128, H], F32, tag="sq")
                ssq = stats.tile([128, 1], F32, tag="ssq")
                nc.scalar.activation(
                    out=sq, in_=src, func=mybir.ActivationFunctionType.Square,
                    accum_out=ssq,
                )
                rstd = stats.tile([128, 1], F32, tag="rstd")
                nc.scalar.activation(
                    out=rstd, in_=ssq, func=mybir.ActivationFunctionType.Sqrt,
                    bias=eps_sb, scale=1.0 / H,
                )
                nc.vector.reciprocal(out=rstd, in_=rstd)
                qn = work.tile([128, H], F32, tag="qn")
                nc.vector.tensor_scalar_mul(qn, src, rstd)
                # rope: h1 = qn1*A - qn2*B ; h2 = qn2*C + qn1*D
                t1 = work.tile([128, HH], F32, tag="t1")
                t2 = work.tile([128, HH], F32, tag="t2")
                qb = work.tile([128, H], BF16, tag="qb")
                nc.vector.tensor_mul(t1, qn[:, 0:HH], rtab[:, 0, :])
                nc.vector.tensor_mul(t2, qn[:, HH:H], rtab[:, 1, :])
                nc.vector.tensor_sub(qb[:, 0:HH], t1, t2)
                t3 = work.tile([128, HH], F32, tag="t1")
                t4 = work.tile([128, HH], F32, tag="t2")
                nc.vector.tensor_mul(t3, qn[:, HH:H], rtab[:, 2, :])
                nc.vector.tensor_mul(t4, qn[:, 0:HH], rtab[:, 3, :])
                nc.vector.tensor_add(qb[:, HH:H], t3, t4)
                nc.sync.dma_start_transpose(
                    out=dest_T[:, dcol, lb * 128:(lb + 1) * 128], in_=qb
                )

            for lb in range(n_lb):
                # slab-sequential so the "proj" tag only needs 2 PSUM banks
                pqs = []
                for s in range(n_wslab):
                    pq = psum.tile([128, 512], F32, tag="proj", bufs=2,
                                   name=f"proj_{lb}_{s}")
                    for dc in range(n_dc):
                        nc.tensor.matmul(
                            pq,
                            xT_sb[:, dc, lb * 128:(lb + 1) * 128],
                            wqkv_sb[:, dc, s * 512:(s + 1) * 512],
                            start=(dc == 0), stop=(dc == n_dc - 1),
                        )
                    pqs.append(pq)

                def pcol(c):  # (tile, offset) for projection column c
                    return pqs[c // 512], c % 512

                rq = ropetab.tile([128, 4, HH], F32, tag="rq")
                nc.sync.dma_start(
                    out=rq,
                    in_=ropeq_d[:, lb * 128:(lb + 1) * 128, :].rearrange(
                        "a p j -> p a j"
                    ),
                )
                rk = ropetab.tile([128, 4, HH], F32, tag="rk")
                nc.sync.dma_start(
                    out=rk,
                    in_=ropek_d[:, lb * 128:(lb + 1) * 128, :].rearrange(
                        "a p j -> p a j"
                    ),
                )
                for qh in range(nq):
                    t, off = pcol(qh * H)
                    norm_rope_head(t, off, rq, qT_sb, qh, lb)
                for kh in range(nkv):
                    t, off = pcol(QCOLS + kh * H)
                    norm_rope_head(t, off, rk, kT_sb, kh, lb)
                # v: plain copy-cast to SBUF
                t, off = pcol(QCOLS + KCOLS)
                nc.vector.tensor_copy(
                    out=v_sb[:, lb, :], in_=t[:, off:off + KCOLS]
                )

            # ---- phase C: attention, fully transposed layout ----
            for qh in range(nq):
                kv = qh // (nq // nkv)
                for lqb in range(n_lqb):
                    l0 = lqb * 512
                    n_sc = min(n_lb, (l0 + 512) // 128) if causal else n_lb
                    pav = psum.tile([128, 512], F32, tag="av", bufs=2,
                                    name=f"av_{qh}_{lqb}")
                    pden = psum.tile([1, 512], F32, tag="den", bufs=2,
                                     name=f"den_{qh}_{lqb}")
                    if not causal:
                        mrows = maskp.tile([128, n_lb, 512], BF16, tag="mask")
                        nc.sync.dma_start(
                            out=mrows,
                            in_=maskT_d[:, l0:l0 + 512].rearrange(
                                "(sb p) l -> p sb l", p=128
                            ),
                        )
                    exps = []

                    def av_den(ex, sc):
                        nc.tensor.matmul(
                            pav, v_sb[:, sc, kv * H:(kv + 1) * H], ex,
                            start=(sc == 0), stop=(sc == n_sc - 1),
                        )
                        nc.tensor.matmul(
                            pden, ones_sb, ex,
                            start=(sc == 0), stop=(sc == n_sc - 1),
                        )

                    for sc in range(n_sc):
                        ps = psum.tile([128, 512], F32, tag="scores", bufs=2,
                                       name=f"sc_{qh}_{lqb}_{sc}")
                        nc.tensor.matmul(
                            ps,
                            kT_sb[:, kv, sc * 128:(sc + 1) * 128],
                            qT_sb[:, qh, l0:l0 + 512],
                            start=True, stop=True,
                        )
                        ex = expp.tile([128, 512], BF16, tag="exp")
                        nc.scalar.activation(
                            out=ex, in_=ps,
                            func=mybir.ActivationFunctionType.Exp, scale=SCALE,
                        )
                        if causal and sc * 128 > l0 - 128:
                            # keep where s0+p <= l0+f
                            nc.gpsimd.affine_select(
                                out=ex, in_=ex, pattern=[[1, 512]],
                                compare_op=mybir.AluOpType.is_ge, fill=0.0,
                                base=l0 - sc * 128, channel_multiplier=-1,
                            )
                        if not causal:
                            nc.vector.tensor_mul(ex, ex, mrows[:, sc, :])
                        exps.append(ex)
                        if sc >= 1:
                            av_den(exps[sc - 1], sc - 1)
                    av_den(exps[n_sc - 1], n_sc - 1)

                    rden = stats.tile([1, 512], F32, tag="rden")
                    nc.vector.reciprocal(out=rden, in_=pden)
                    # partition-broadcast via DRAM bounce (SBUF APs cannot
                    # have a zero partition step; DRAM APs can)
                    rden_dr = dramp.tile([512], F32, tag="rdendr")
                    nc.sync.dma_start(out=rden_dr, in_=rden)
                    rdenb = work.tile([128, 512], F32, tag="rdenb")
                    nc.sync.dma_start(
                        out=rdenb,
                        in_=bass.AP(
                            tensor=rden_dr.tensor, offset=rden_dr.offset,
                            ap=[[0, 128], [1, 512]],
                        ),
                    )
                    nc.vector.tensor_mul(
                        qkvT_sb[:, qh, l0:l0 + 512], pav, rdenb
                    )

            # ---- phase D: output projection ----
            for lb in range(n_lb):
                for db in range(D_ // 512):
                    po = psum.tile([128, 512], F32, tag="scores", bufs=2,
                                   name=f"po_{lb}_{db}")
                    for hh in range(nq):
                        nc.tensor.matmul(
                            po,
                            qkvT_sb[:, hh, lb * 128:(lb + 1) * 128],
                            wo_sb[:, hh, db * 512:(db + 1) * 512],
                            start=(hh == 0), stop=(hh == nq - 1),
                        )
                    ot = outp.tile([128, 512], F32, tag="ot")
                    nc.vector.tensor_copy(out=ot, in_=po)
                    nc.sync.dma_start(
                        out=out_d[lb * 128:(lb + 1) * 128, db * 512:(db + 1) * 512],
                        in_=ot,
                    )
    return nc


# ---------------- host side ----------------

def _rope_tables(pos, norm_w):
    """A,B,C,D [4, L, H/2] f32 with the rms-norm weight folded in.
    h1 = q1*A - q2*B ; h2 = q2*C + q1*D  (q already divided by rms)."""
    hh = H // 2
    fraction = 2.0 * np.arange(hh, dtype=np.float32) / np.float32(H)
    timescale = np.float32(ROPE_THETA) ** fraction
    sinusoid = pos.astype(np.float32)[:, None] / timescale[None, :]
    sin = np.sin(sinusoid).astype(np.float32)
    cos = np.cos(sinusoid).astype(np.float32)
    w1 = norm_w[:hh].astype(np.float32)
    w2 = norm_w[hh:].astype(np.float32)
    return np.stack([cos * w1, sin * w2, cos * w2, sin * w1]).astype(np.float32)


_KERNELS = {}
TRACE = False
LAST_RESULTS = None


def _get_kernel(causal):
    if causal not in _KERNELS:
        _KERNELS[causal] = build_core_kernel(causal=causal)
    return _KERNELS[causal]


def kernel(**inputs):
    x = np.asarray(inputs["x"], dtype=np.float32)
    pos = np.asarray(inputs["position_ids"])
    mask = np.asarray(inputs["attn_mask"]).astype(bool)
    wq = np.asarray(inputs["wq"], dtype=np.float32)
    wk = np.asarray(inputs["wk"], dtype=np.float32)
    wv = np.asarray(inputs["wv"], dtype=np.float32)
    wo = np.asarray(inputs["wo"], dtype=np.float32)
    qw = np.asarray(inputs["q_norm_w"], dtype=np.float32)
    kw = np.asarray(inputs["k_norm_w"], dtype=np.float32)

    tril = np.tril(np.ones((L, L), dtype=bool))
    causal = all(np.array_equal(mask[b], tril) for b in range(B))
    nc = _get_kernel(causal)

    bf = ml_dtypes.bfloat16
    per_batch = []
    for b in range(B):
        d = {
            "xT": np.ascontiguousarray(x[b].T).astype(bf),
            "ropeq": _rope_tables(pos[b], qw),
            "ropek": _rope_tables(pos[b], kw),
        }
        if not causal:
            d["maskT"] = np.ascontiguousarray(mask[b].T).astype(bf)
        per_batch.append(d)

    in_maps = []
    for c in range(N_CORES):
        b, g = divmod(c, N_CORES // B)
        qs = slice(QH_PER_CORE * g, QH_PER_CORE * (g + 1))
        ks = slice(KV_PER_CORE * g, KV_PER_CORE * (g + 1))
        wqkv = np.concatenate(
            [
                wq[:, qs, :].reshape(D, QH_PER_CORE * H),
                wk[:, ks, :].reshape(D, KV_PER_CORE * H),
                wv[:, ks, :].reshape(D, KV_PER_CORE * H),
            ],
            axis=1,
        ).astype(bf)
        m = dict(per_batch[b])
        m["wqkv"] = wqkv
        m["wo"] = np.ascontiguousarray(wo[qs].reshape(QH_PER_CORE * H, D)).astype(bf)
        in_maps.append(m)

    global LAST_RESULTS
    res = run_bass_kernel_spmd(
        nc, in_maps, core_ids=list(range(N_CORES)), trace=TRACE
    )
    LAST_RESULTS = res
    out = np.zeros((B, L, D), dtype=np.float32)
    for c in range(N_CORES):
        out[c // (N_CORES // B)] += res.results[c]["out"]
    return out
